# revision 1
# baseline (speedup 1.0000x reference)
"""Trainium2 Bass kernel for nn_BiLSTMTagger: embeddings -> 2-layer BiLSTM
-> biaffine scoring -> row softmax.

Self-contained: takes the full (unsharded) inputs, runs an SPMD bass kernel
on 8 NeuronCores, returns the full [S, S] float32 output.

Strategy: the 2048-step sequential LSTM recurrence dominates and cannot be
parallelized across cores, so the whole network is computed replicated on
every core (SPMD-identical program; core 0's output is returned).  Within a
core, the fwd and bwd LSTM streams are interleaved so the tensor engine's
recurrent matvec of one direction overlaps the vector/scalar gate math of
the other.

Layouts (all host-prepped):
  - Gates reordered (i, f, o, g) with the g block's weights/bias pre-scaled
    by 2 so tanh(x) = 2*sigmoid(2x) - 1 needs only one sigmoid pass.
  - All state is column-major: vector v[d] lives at partition d%128,
    column d//128, so elementwise gate math uses all 128 lanes.
  - W_hh tiles are stationary [K=128, M=128] fp16 matmul operands; h is the
    [128, 1] moving operand; pre-activations are injected into PSUM with an
    identity-matmul so the whole gate vector accumulates in one PSUM bank.
"""

import os
import numpy as np
from contextlib import ExitStack

S = 1024
D = 1024
H = 512
G = 4 * H
EW, EP = 896, 128
VW, VP = 50000, 50
L = 2

_BUILD_CACHE = {}


def _dt():
    import concourse.mybir as mybir

    return mybir


def build(s=S, whh_fp8=False, warm=0, warm_n=256):
    key = (s, whh_fp8, warm, warm_n)
    if key in _BUILD_CACHE:
        return _BUILD_CACHE[key]

    import concourse.bass as bass
    import concourse.mybir as mybir
    import concourse.tile as tile
    from concourse import bacc

    f16 = mybir.dt.float16
    f32 = mybir.dt.float32
    u32 = mybir.dt.uint32
    whh_dt = mybir.dt.float8e4 if whh_fp8 else f16
    AF = mybir.ActivationFunctionType
    OP = mybir.AluOpType

    TG = s // 128          # token groups of 128 (also score row-chunks)
    CH = min(s, 512)       # free-dim chunk for GEMMs (one PSUM bank)
    NCH = (s + CH - 1) // CH

    nc = bacc.Bacc("TRN2", target_bir_lowering=False, debug=False, num_devices=8)

    tok_w = nc.dram_tensor("tok_w", [128, TG], u32, kind="ExternalInput").ap()
    tok_p = nc.dram_tensor("tok_p", [128, TG], u32, kind="ExternalInput").ap()
    wemb = nc.dram_tensor("wemb", [VW, EW], f16, kind="ExternalInput").ap()
    pemb = nc.dram_tensor("pemb", [VP, EP], f16, kind="ExternalInput").ap()
    wiT = nc.dram_tensor("wiT", [L, 2, 128, 8, G], f16, kind="ExternalInput").ap()
    whhT = nc.dram_tensor("whhT", [L, 2, 128, 4, G], whh_dt, kind="ExternalInput").ap()
    bcm = nc.dram_tensor("bcm", [L, 2, 128, 16], f32, kind="ExternalInput").ap()
    whTd = nc.dram_tensor("whTd", [128, 8, D], f16, kind="ExternalInput").ap()
    wdTd = nc.dram_tensor("wdTd", [128, 8, D], f16, kind="ExternalInput").ap()
    wbid = nc.dram_tensor("wbid", [128, 8, D], f16, kind="ExternalInput").ap()
    bhd = nc.dram_tensor("bhd", [128, 8], f32, kind="ExternalInput").ap()
    bdd = nc.dram_tensor("bdd", [128, 8], f32, kind="ExternalInput").ap()
    idf16 = nc.dram_tensor("idf16", [128, 128], f16, kind="ExternalInput").ap()
    out = nc.dram_tensor("out", [s, s], f32, kind="ExternalOutput").ap()

    with tile.TileContext(nc) as tc:
        with ExitStack() as ctx:
            work = ctx.enter_context(tc.tile_pool(name="work", bufs=1))
            psum = ctx.enter_context(tc.tile_pool(name="psum", bufs=1, space="PSUM"))
            dram = ctx.enter_context(tc.tile_pool(name="dram", bufs=1, space="DRAM"))

            _ps_alt = [0]

            def ps_tile(shape=(128, CH), dtype=f32):
                # rotate between the two 4-slot bank groups
                _ps_alt[0] ^= 1
                return psum.tile(list(shape), dtype,
                                 tag="gpP" if _ps_alt[0] else "gpQ",
                                 bufs=4, name="ps")

            # ---------- persistent SBUF ----------
            id_sb = work.tile([128, 128], f16, name="id_sb")
            nc.sync.dma_start(id_sb[:], idf16)
            tokw_sb = work.tile([128, TG], u32, name="tokw_sb")
            nc.sync.dma_start(tokw_sb[:], tok_w)
            tokp_sb = work.tile([128, TG], u32, name="tokp_sb")
            nc.sync.dma_start(tokp_sb[:], tok_p)
            b_sb = work.tile([128, L * 2 * 16], f32, name="b_sb")
            for l in range(L):
                for d in range(2):
                    nc.sync.dma_start(
                        b_sb[:, (2 * l + d) * 16:(2 * l + d) * 16 + 16], bcm[l, d]
                    )
            bh_sb = work.tile([128, 8], f32, name="bh_sb")
            nc.sync.dma_start(bh_sb[:], bhd)
            bd_sb = work.tile([128, 8], f32, name="bd_sb")
            nc.sync.dma_start(bd_sb[:], bdd)

            whh_sb = {}
            for l in range(L):
                for d in range(2):
                    t = work.tile([128, 4, G], whh_dt, name=f"whh_{l}{d}")
                    nc.sync.dma_start(t[:], whhT[l, d])
                    whh_sb[(l, d)] = t

            hs = {}
            for l in range(L):
                for d in range(2):
                    hs[(l, d)] = work.tile([128, s, 4], f16, name=f"hs_{l}{d}")

            # preT scratch in DRAM, one per layer: [dir, 128, 16, s] fp16
            preT_dram = [
                dram.tile([2, 128, 16, s], f16, name=f"preT_l{l}") for l in range(L)
            ]

            # ---------- embeddings: gather + transpose ----------
            # embT[p, j, t] = emb_dim(128j + p) of token t;  j<7 word, j=7 pos
            embT = work.tile([128, 8, s], f16, tag="t16", name="embT")
            for g in range(TG):
                ewg = work.tile([128, EW], f16, tag="ewg", bufs=2, name="ewg")
                nc.gpsimd.indirect_dma_start(
                    out=ewg[:],
                    out_offset=None,
                    in_=wemb,
                    in_offset=bass.IndirectOffsetOnAxis(ap=tokw_sb[:, g:g + 1], axis=0),
                )
                epg = work.tile([128, EP], f16, tag="epg", bufs=2, name="epg")
                nc.gpsimd.indirect_dma_start(
                    out=epg[:],
                    out_offset=None,
                    in_=pemb,
                    in_offset=bass.IndirectOffsetOnAxis(ap=tokp_sb[:, g:g + 1], axis=0),
                )
                for j in range(8):
                    tps = ps_tile((128, 128), f16)
                    src = ewg[:, j * 128:(j + 1) * 128] if j < 7 else epg[:]
                    nc.tensor.transpose(tps[:], src, id_sb[:])
                    dst = embT[:, j, g * 128:(g + 1) * 128]
                    if j % 2 == 0:
                        nc.scalar.copy(dst, tps[:])
                    else:
                        nc.vector.tensor_copy(dst, tps[:])

            # ---------- helpers ----------
            def pre_gemm(l, d, rhs_chunks):
                """Compute preT_dram[l][d][:, m, :] = (Wi_r @ x.T + b) col-major.

                rhs_chunks: function (j, h) -> AP [128, CH] giving the K-chunk
                j of the layer input, t-columns [h*CH, (h+1)*CH).
                """
                wi_sb = work.tile([128, 8, G], f16, tag="t32", name="wi_sb")
                nc.sync.dma_start(wi_sb[:], wiT[l, d])
                for m in range(16):
                    for h in range(NCH):
                        pp = ps_tile()
                        for j in range(8):
                            nc.tensor.matmul(
                                pp[:],
                                wi_sb[:, j, m * 128:(m + 1) * 128],
                                rhs_chunks(j, h),
                                start=(j == 0),
                                stop=(j == 7),
                            )
                        stg = work.tile([128, CH], f16, tag="stg", bufs=4, name="stg")
                        nc.scalar.activation(
                            stg[:], pp[:], AF.Identity,
                            bias=b_sb[:, (2 * l + d) * 16 + m:(2 * l + d) * 16 + m + 1],
                        )
                        nc.sync.dma_start(
                            preT_dram[l][d, :, m, h * CH:(h + 1) * CH], stg[:]
                        )

            junk_ps = (
                psum.tile([128, warm_n], f32, tag="junk", bufs=1, name="junk_ps")
                if warm
                else None
            )

            def pe_warm(l, d):
                # dependency-free matmuls that keep the PE HAM at K=8/8
                # while the gate chain runs on ACT/DVE
                for _ in range(warm):
                    nc.tensor.matmul(
                        junk_ps[:],
                        whh_sb[(l, d)][:, 0, 0:128],
                        whh_sb[(l, d)][:, 1, 0:warm_n],
                        start=True,
                        stop=True,
                    )

            def recurrence(l):
                """Software-pipelined fwd/bwd LSTM over s steps for layer l.

                Emission order per step: fwd matvec | bwd gate-chain(prev) |
                bwd matvec | fwd gate-chain — so each direction's serial gate
                math hides under the other direction's PE matvec.
                """
                c_prev = [None, None]
                pch = [None, None]
                pend = [None, None]  # (gp, tau) awaiting gate chain

                pnext = [None, None]

                def load_chunk(d, c0):
                    t = work.tile(
                        [128, 16, 128], f16, tag=f"pch{d}", bufs=2, name=f"pch{d}"
                    )
                    nc.sync.dma_start(
                        t[:], preT_dram[l][d, :, :, c0 * 128:(c0 + 1) * 128]
                    )
                    return t

                def matvec(d, step):
                    tau = step if d == 0 else s - 1 - step
                    if step == 0:
                        pch[d] = load_chunk(d, tau // 128)
                    elif step % 128 == 0:
                        pch[d] = pnext[d]
                    elif step % 128 == 64 and step + 64 < s:
                        nxt = step // 128 + 1
                        pnext[d] = load_chunk(d, nxt if d == 0 else TG - 1 - nxt)
                    ti = tau % 128
                    # gate order is (g~, i, f, o): chunks 0-11 go to gpP so
                    # their sigmoid (and the c update) can start while the
                    # o-gate chunks 12-15 still accumulate into gpQ.
                    gpP = psum.tile([128, CH], f32, tag="gpP", bufs=4, name="gpP")
                    gpQ = psum.tile([128, CH], f32, tag="gpQ", bufs=4, name="gpQ")
                    nc.tensor.matmul(
                        gpP[:, 0:12], id_sb[:], pch[d][:, 0:12, ti:ti + 1],
                        start=True, stop=(step == 0),
                    )
                    nc.tensor.matmul(
                        gpQ[:, 0:4], id_sb[:], pch[d][:, 12:16, ti:ti + 1],
                        start=True, stop=(step == 0),
                    )
                    if step > 0:
                        tprev = tau - 1 if d == 0 else tau + 1
                        for m in range(16):
                            out = gpP[:, m:m + 1] if m < 12 else gpQ[:, m - 12:m - 11]
                            for k in range(4):
                                nc.tensor.matmul(
                                    out,
                                    whh_sb[(l, d)][:, k, m * 128:(m + 1) * 128],
                                    hs[(l, d)][:, tprev, k:k + 1],
                                    start=False,
                                    stop=(m, k) in ((11, 3), (15, 3)),
                                )
                    pend[d] = (gpP, gpQ, tau, step)

                def chain(d):
                    gpP, gpQ, tau, step = pend[d]
                    st = work.tile([128, 12], f32, tag=f"st{d}", bufs=4,
                                   name=f"st{d}")
                    nc.scalar.activation(st[:], gpP[:, 0:12], AF.Sigmoid)
                    cn = work.tile([128, 4], f32, tag=f"cn{d}", bufs=4,
                                   name=f"cn{d}")
                    ut = work.tile([128, 4], f32, tag=f"ut{d}", bufs=4,
                                   name=f"ut{d}")
                    # u = (sig_g - 0.5) * sig_i  == tanh(g)/2 * sig_i
                    nc.vector.scalar_tensor_tensor(
                        out=ut[:], in0=st[:, 0:4], scalar=-0.5,
                        in1=st[:, 4:8], op0=OP.add, op1=OP.mult,
                    )
                    if step == 0:
                        nc.vector.tensor_scalar_mul(cn[:], ut[:], 2.0)
                    else:
                        m2 = work.tile([128, 4], f32, tag=f"m2{d}", bufs=4,
                                       name=f"m2{d}")
                        nc.vector.tensor_tensor(
                            out=m2[:], in0=st[:, 8:12], in1=c_prev[d][:],
                            op=OP.mult,
                        )
                        nc.vector.scalar_tensor_tensor(
                            out=cn[:], in0=ut[:], scalar=2.0, in1=m2[:],
                            op0=OP.mult, op1=OP.add,
                        )
                    # sigmoid(o) for the late gpQ chunks
                    so = work.tile([128, 4], f32, tag=f"so{d}", bufs=4,
                                   name=f"so{d}")
                    nc.scalar.activation(so[:], gpQ[:, 0:4], AF.Sigmoid)
                    # tanh lands in spare columns of gpQ's PSUM bank:
                    # ScalarE->PSUM is the fast ACT destination, and the
                    # following tensor_tensor reads it back as psum-src.
                    tct = gpQ[:, 32:36]
                    nc.scalar.activation(tct, cn[:], AF.Tanh)
                    nc.vector.tensor_tensor(
                        out=hs[(l, d)][:, tau, :], in0=so[:],
                        in1=tct, op=OP.mult,
                    )
                    c_prev[d] = cn

                for step in range(s):
                    matvec(0, step)
                    if step > 0:
                        chain(1)
                    matvec(1, step)
                    chain(0)
                chain(1)

            # ---------- layer 1 ----------
            pre_gemm(0, 0, lambda j, h: embT[:, j, h * CH:(h + 1) * CH])
            pre_gemm(0, 1, lambda j, h: embT[:, j, h * CH:(h + 1) * CH])
            recurrence(0)

            # ---------- layer 2 ----------
            def l2_rhs(j, h):
                src = hs[(0, 0)] if j < 4 else hs[(0, 1)]
                return src[:, h * CH:(h + 1) * CH, j % 4]

            pre_gemm(1, 0, l2_rhs)
            pre_gemm(1, 1, l2_rhs)
            recurrence(1)

            # ---------- tail: head/dep projections, biaffine, softmax ----------
            def hs2_rhs(j, h):
                src = hs[(1, 0)] if j < 4 else hs[(1, 1)]
                return src[:, h * CH:(h + 1) * CH, j % 4]

            # headT[e, r] then aT[e, r] share one 32KB/partition slot
            tailAB = work.tile([128, 16, s], f16, tag="t32", name="tailAB")
            depT = work.tile([128, 8, s], f16, tag="t16", name="depT")

            wq = work.tile([128, 8, D], f16, tag="twq", name="wq_h")
            nc.sync.dma_start(wq[:], whTd)
            for e in range(8):
                for h in range(NCH):
                    pp = ps_tile()
                    for j in range(8):
                        nc.tensor.matmul(
                            pp[:], wq[:, j, e * 128:(e + 1) * 128], hs2_rhs(j, h),
                            start=(j == 0), stop=(j == 7),
                        )
                    nc.scalar.activation(
                        tailAB[:, e, h * CH:(h + 1) * CH], pp[:], AF.Identity,
                        bias=bh_sb[:, e:e + 1],
                    )

            wq2 = work.tile([128, 8, D], f16, tag="twq", name="wq_d")
            nc.sync.dma_start(wq2[:], wdTd)
            for e in range(8):
                for h in range(NCH):
                    pp = ps_tile()
                    for j in range(8):
                        nc.tensor.matmul(
                            pp[:], wq2[:, j, e * 128:(e + 1) * 128], hs2_rhs(j, h),
                            start=(j == 0), stop=(j == 7),
                        )
                    nc.scalar.activation(
                        depT[:, e, h * CH:(h + 1) * CH], pp[:], AF.Identity,
                        bias=bd_sb[:, e:e + 1],
                    )

            wq3 = work.tile([128, 8, D], f16, tag="twq", name="wq_bi")
            nc.sync.dma_start(wq3[:], wbid)
            for e in range(8):
                for h in range(NCH):
                    pp = ps_tile()
                    for j in range(8):
                        nc.tensor.matmul(
                            pp[:], wq3[:, j, e * 128:(e + 1) * 128],
                            tailAB[:, j, h * CH:(h + 1) * CH],
                            start=(j == 0), stop=(j == 7),
                        )
                    nc.vector.tensor_copy(
                        tailAB[:, 8 + e, h * CH:(h + 1) * CH], pp[:]
                    )

            # scores rows (128 at a time) + softmax
            for r in range(TG):
                sps = []
                for h in range(NCH):
                    pp = ps_tile()
                    for e in range(8):
                        nc.tensor.matmul(
                            pp[:], tailAB[:, 8 + e, r * 128:(r + 1) * 128],
                            depT[:, e, h * CH:(h + 1) * CH],
                            start=(e == 0), stop=(e == 7),
                        )
                    sps.append(pp)
                mx = work.tile([128, NCH], f32, tag="mx", bufs=2, name="mx")
                for h in range(NCH):
                    nc.vector.tensor_reduce(
                        mx[:, h:h + 1], sps[h][:], axis=mybir.AxisListType.X,
                        op=OP.max,
                    )
                nmx = work.tile([128, 1], f32, tag="nmx", bufs=2, name="nmx")
                if NCH > 1:
                    nc.vector.tensor_reduce(
                        nmx[:], mx[:], axis=mybir.AxisListType.X, op=OP.max
                    )
                    nc.vector.tensor_scalar_mul(nmx[:], nmx[:], -1.0)
                else:
                    nc.vector.tensor_scalar_mul(nmx[:], mx[:], -1.0)
                esb = work.tile([128, s], f32, tag="esb", bufs=2, name="esb")
                ssum = work.tile([128, NCH], f32, tag="ssum", bufs=2, name="ssum")
                for h in range(NCH):
                    nc.scalar.activation(
                        esb[:, h * CH:(h + 1) * CH], sps[h][:], AF.Exp,
                        bias=nmx[:], accum_out=ssum[:, h:h + 1],
                    )
                rsum = work.tile([128, 1], f32, tag="rsum", bufs=2, name="rsum")
                if NCH > 1:
                    nc.vector.tensor_reduce(
                        rsum[:], ssum[:], axis=mybir.AxisListType.X, op=OP.add
                    )
                    nc.vector.reciprocal(rsum[:], rsum[:])
                else:
                    nc.vector.reciprocal(rsum[:], ssum[:])
                osb = work.tile([128, s], f32, tag="osb", bufs=2, name="osb")
                nc.vector.tensor_scalar(
                    out=osb[:], in0=esb[:], scalar1=rsum[:], scalar2=None,
                    op0=OP.mult,
                )
                nc.sync.dma_start(out[r * 128:(r + 1) * 128, :], osb[:])

    nc.compile()
    if os.environ.get("KERNEL_NO_THIN") != "1":
        _thin_pe_incs(nc)
    _BUILD_CACHE[key] = nc
    return nc


def _thin_pe_incs(nc):
    """Strip per-matmul semaphore increments from the PE stream.

    Tile gives every PE instruction a +1 update on the PE progress
    semaphore; the EVT_SEM writes serialize at ~30-45ns each, which paces
    the whole recurrence (the gate sigmoid waits on a cumulative count
    that lags instruction issue by the backlog).  Only a handful of
    values per step are ever waited on, so keep exactly the increments
    that waiting thresholds reference and renumber the thresholds.
    """
    import concourse.mybir as mybir
    from collections import Counter
    from bisect import bisect_left

    f = nc.m.functions[0]
    blocks = f.blocks

    # 1. identify the PE progress semaphore (most common PE sem-inc id)
    cnt = Counter()
    for blk in blocks:
        for i in blk.instructions:
            if getattr(i, "engine", None) != mybir.EngineType.PE:
                continue
            si = getattr(i, "sync_info", None)
            if si is None:
                continue
            for u in si.on_update:
                if u.update_mode == "sem-inc" and u.update_reg is None:
                    cnt[u.id] += 1
    if not cnt:
        return
    sem_id, total = cnt.most_common(1)[0]
    if total < 1000:
        return

    # 2. collect every wait threshold on that semaphore (any engine)
    thresholds = set()
    for blk in blocks:
        for i in blk.instructions:
            si = getattr(i, "sync_info", None)
            if si is None:
                continue
            for w in si.on_wait:
                if w.id == sem_id:
                    if w.wait_mode != "sem-ge-imm" or w.wait_value is None:
                        return  # unexpected form; abort surgery
                    thresholds.add(w.wait_value)

    # 3. walk PE instructions in execution order, assign ordinals, keep
    #    only increments whose ordinal is a referenced threshold
    kept = []
    ordinal = 0
    for blk in blocks:
        for i in blk.instructions:
            if getattr(i, "engine", None) != mybir.EngineType.PE:
                continue
            si = getattr(i, "sync_info", None)
            if si is None:
                continue
            ups = list(si.on_update)
            hit = [u for u in ups if u.id == sem_id and u.update_mode == "sem-inc"]
            if not hit:
                continue
            assert len(hit) == 1 and hit[0].update_value == 1
            ordinal += 1
            if ordinal in thresholds or ordinal == total:
                kept.append(ordinal)
            else:
                i.sync_info = mybir.SyncInfo(
                    on_wait=list(si.on_wait),
                    on_update=[u for u in ups if u is not hit[0]],
                )
    assert ordinal == total
    for t in thresholds:
        assert 1 <= t <= total, t

    # 4. renumber waits: value V -> rank of V among kept ordinals
    for blk in blocks:
        for i in blk.instructions:
            si = getattr(i, "sync_info", None)
            if si is None:
                continue
            if not any(w.id == sem_id for w in si.on_wait):
                continue
            new_waits = []
            for w in si.on_wait:
                if w.id == sem_id:
                    idx = bisect_left(kept, w.wait_value)
                    assert idx < len(kept) and kept[idx] == w.wait_value
                    w = mybir.SyncWait(
                        sync_type=w.sync_type,
                        id=w.id,
                        ant_name=w.ant_name,
                        wait_mode=w.wait_mode,
                        wait_value=idx + 1,
                        wait_reg=None,
                    )
                new_waits.append(w)
            i.sync_info = mybir.SyncInfo(
                on_wait=new_waits, on_update=list(si.on_update)
            )


def _prep_inputs(inputs, s=S, whh_fp8=False):
    import ml_dtypes

    x = np.asarray(inputs["x"]).reshape(-1)[:s].astype(np.uint32)
    xp = np.asarray(inputs["x_pos"]).reshape(-1)[:s].astype(np.uint32)
    we = np.asarray(inputs["word_emb"], dtype=np.float32).astype(np.float16)
    pe = np.asarray(inputs["pos_emb"], dtype=np.float32).astype(np.float16)
    Wih = np.asarray(inputs["W_ih"], dtype=np.float32)
    Whh = np.asarray(inputs["W_hh"], dtype=np.float32)
    b = np.asarray(inputs["b_lstm"], dtype=np.float32)
    Wh = np.asarray(inputs["Wh"], dtype=np.float32)
    bh = np.asarray(inputs["bh"], dtype=np.float32)
    Wd = np.asarray(inputs["Wd"], dtype=np.float32)
    bd = np.asarray(inputs["bd"], dtype=np.float32)
    Wbi = np.asarray(inputs["Wbi"], dtype=np.float32)

    TG = s // 128
    # reorder gates (i, f, g, o) -> (g, i, f, o); scale the g block by 2
    perm = np.concatenate(
        [np.arange(2 * H, 3 * H), np.arange(0, H), np.arange(H, 2 * H),
         np.arange(3 * H, 4 * H)]
    )
    sc = np.ones((G, 1), np.float32)
    sc[:H] = 2.0
    Wir = Wih[:, :, perm, :] * sc[None, None]
    Whr = Whh[:, :, perm, :] * sc[None, None]
    br = b[:, :, perm] * sc[None, None, :, 0]

    wiT = np.ascontiguousarray(
        Wir.transpose(0, 1, 3, 2).reshape(L, 2, 8, 128, G).transpose(0, 1, 3, 2, 4)
    ).astype(np.float16)
    whhT_f = np.ascontiguousarray(
        Whr.transpose(0, 1, 3, 2).reshape(L, 2, 4, 128, G).transpose(0, 1, 3, 2, 4)
    )
    if whh_fp8:
        whhT = whhT_f.astype(ml_dtypes.float8_e4m3fn)
    else:
        whhT = whhT_f.astype(np.float16)
    bcm = np.ascontiguousarray(br.reshape(L, 2, 16, 128).transpose(0, 1, 3, 2))

    whT = np.ascontiguousarray(Wh.T.reshape(8, 128, D).transpose(1, 0, 2)).astype(
        np.float16
    )
    wdT = np.ascontiguousarray(Wd.T.reshape(8, 128, D).transpose(1, 0, 2)).astype(
        np.float16
    )
    wbi = np.ascontiguousarray(Wbi.reshape(8, 128, D).transpose(1, 0, 2)).astype(
        np.float16
    )
    bh_cm = np.ascontiguousarray(bh.reshape(8, 128).T)
    bd_cm = np.ascontiguousarray(bd.reshape(8, 128).T)

    return {
        "tok_w": np.ascontiguousarray(x.reshape(TG, 128).T),
        "tok_p": np.ascontiguousarray(xp.reshape(TG, 128).T),
        "wemb": we,
        "pemb": pe,
        "wiT": wiT,
        "whhT": whhT,
        "bcm": bcm,
        "whTd": whT,
        "wdTd": wdT,
        "wbid": wbi,
        "bhd": bh_cm,
        "bdd": bd_cm,
        "idf16": np.eye(128, dtype=np.float16),
    }


def run(inputs, s=S, whh_fp8=False, trace=False, tmpdir=None):
    from concourse import bass_utils

    nc = build(s, whh_fp8)
    in_map = _prep_inputs(inputs, s, whh_fp8)
    in_maps = [in_map for _ in range(8)]
    res = bass_utils.run_bass_kernel_spmd(
        nc, in_maps, core_ids=list(range(8)), trace=trace, tmpdir=tmpdir
    )
    return res


def kernel(**inputs) -> np.ndarray:
    res = run(inputs)
    return np.asarray(res.results[0]["out"], dtype=np.float32)

